# revision 23
# baseline (speedup 1.0000x reference)
"""Trainium2 Bass kernel for nn_DemeanedGlobalLossD.

Strategy: shard the feature dim D=73728 over 8 cores (9216 each). Each core
computes the upper block-rows of a partial Gram matrix G_c = Z_c @ Z_c.T
([384,384], bf16 matmuls, fp32 accum) of the demeaned features; one bf16
AllReduce sums the partials and PE transposes mirror the lower blocks
(G is exactly symmetric). The row norms are sqrt(diag(G)), so the
normalize step needs no extra reduction. The loss tail (cosine scaling,
exp, masked negative sums, pair log terms) runs replicated on every core
on the tiny 384x384 matrix.

The 4 pair families of the reference all use partner offsets +16 / +192
(mod 384), and S is symmetric, so the 768 pair terms reduce to elementwise
logs over three shifted diagonals of E = exp(S/T):
  total = sum_i [ 2*log(v192+Ds) + log(v16+Ds) + log(v16m+Ds)
                  - 2*log(v192) - 2*log(v16) ](i)
with v192[i]=E[i,(i+192)%384], v16[i]=E[i,(i+16)%384], v16m[i]=E[i,(i-16)%384],
Ds = masked (j%16 != i%16) row sums of E.  loss = total / 576.
"""
import numpy as np
import ml_dtypes

import concourse.bass as bass
import concourse.bacc as bacc
import concourse.tile as tile
import concourse.mybir as mybir
from concourse import bass_utils

F32 = mybir.dt.float32
BF16 = mybir.dt.bfloat16

N = 384
D = 73728
N_CORES = 8
DL = D // N_CORES          # 9216 features per core
KT = DL // 128             # 72 k-tiles per core
CHUNK_KT = (2, 10, 10, 10, 10, 10, 10, 10)   # k-tiles per xt chunk
NCHUNK = len(CHUNK_KT)
PD = 16
TEMP = 0.1
EPS = 1e-6
DENOM = 576.0              # N_TRANSFORMS * 3 * BS

_CACHE = {}
LAST_RESULTS = None


def _build_module():
    nc = bacc.Bacc("TRN2", target_bir_lowering=False, debug=False,
                   num_devices=N_CORES)
    xt_d = nc.dram_tensor("xt", [128, KT * 384], BF16, kind="ExternalInput")
    mt_d = nc.dram_tensor("mt", [128, KT * 16], BF16, kind="ExternalInput")
    eye_d = nc.dram_tensor("eye", [128, 1152], F32, kind="ExternalInput")
    neg_d = nc.dram_tensor("neg", [128, 1152], F32, kind="ExternalInput")
    d192_d = nc.dram_tensor("d192", [128, 1152], F32, kind="ExternalInput")
    d16_d = nc.dram_tensor("d16", [128, 1152], F32, kind="ExternalInput")
    d16m_d = nc.dram_tensor("d16m", [128, 1152], F32, kind="ExternalInput")
    wts_d = nc.dram_tensor("wts", [128, 5], F32, kind="ExternalInput")
    out_d = nc.dram_tensor("out", [1, 1], F32, kind="ExternalOutput")

    AOp = mybir.AluOpType
    AF = mybir.ActivationFunctionType
    chunk_of_k = []
    for ci, n in enumerate(CHUNK_KT):
        chunk_of_k += [ci] * n
    chunk_base = [sum(CHUNK_KT[:ci]) for ci in range(NCHUNK)]

    with tile.TileContext(nc) as tc:
        with (
            tc.tile_pool(name="sb", bufs=1) as sb,
            tc.tile_pool(name="ps", bufs=1, space="PSUM") as ps,
            tc.tile_pool(name="dram", bufs=1, space="DRAM") as dram,
        ):
            # mean via the scalar HWDGE queue so it races ahead of the x
            # chunks on the sync queue; masks follow on the same queue
            # (they are needed only after the AllReduce)
            mt = sb.tile([128, KT * 16], BF16)
            nc.scalar.dma_start(mt[:], mt_d.ap()[:])
            # x chunks as separate tiles for chunk-granular deps
            xtc = []
            for ci in range(NCHUNK):
                cw = CHUNK_KT[ci] * 384
                co = chunk_base[ci] * 384
                x_c = sb.tile([128, cw], BF16, name=f"xt{ci}")
                nc.sync.dma_start(x_c[:], xt_d.ap()[:, co:co + cw])
                xtc.append(x_c)
            eye = sb.tile([128, 1152], F32)
            nc.scalar.dma_start(eye[:], eye_d.ap()[:])
            neg = sb.tile([128, 1152], F32)
            nc.scalar.dma_start(neg[:], neg_d.ap()[:])
            d192 = sb.tile([128, 1152], F32)
            nc.scalar.dma_start(d192[:], d192_d.ap()[:])
            d16 = sb.tile([128, 1152], F32)
            nc.scalar.dma_start(d16[:], d16_d.ap()[:])
            d16m = sb.tile([128, 1152], F32)
            nc.scalar.dma_start(d16m[:], d16m_d.ap()[:])
            wts = sb.tile([128, 5], F32)
            nc.scalar.dma_start(wts[:], wts_d.ap()[:])

            # preload the ACT tables (Sqrt/Exp/Ln) while DMAs stream so the
            # table loads don't land in the post-AllReduce critical path
            tbl = sb.tile([1, 1], F32)
            nc.vector.memset(tbl[:], 1.0)
            nc.scalar.sqrt(tbl[:], tbl[:])
            nc.scalar.activation(tbl[:], tbl[:], AF.Exp)
            nc.scalar.activation(tbl[:], tbl[:], AF.Ln)

            # demean into per-chunk zt tiles (broadcast 24 reps of the 16 means)
            ztc = []
            for ci in range(NCHUNK):
                z_c = sb.tile([128, CHUNK_KT[ci] * 384], BF16, name=f"zt{ci}")
                ztc.append(z_c)
            for k in range(KT):
                ci = chunk_of_k[k]
                kk = k - chunk_base[ci]
                in1 = mt[:, k * 16:(k + 1) * 16].unsqueeze(1).broadcast_to([128, 24, 16])
                o3 = ztc[ci][:, kk * 384:(kk + 1) * 384].rearrange("p (a b) -> p a b", b=16)
                i3 = xtc[ci][:, kk * 384:(kk + 1) * 384].rearrange("p (a b) -> p a b", b=16)
                nc.vector.tensor_tensor(o3, i3, in1, AOp.subtract)

            # partial Gram, upper block-rows only (G is exactly symmetric):
            # m=0 -> abs cols 0:384, m=1 -> 128:384, m=2 -> 256:384.
            # k-outer so PE paces the DMA stream once.
            UPW = (384, 256, 128)          # upper widths per block-row
            UPO = (0, 384, 640)            # offsets in the packed buffer
            psg = [ps.tile([128, UPW[m]], F32, name=f"psg{m}") for m in range(3)]
            for k in range(KT):
                ci = chunk_of_k[k]
                kk = k - chunk_base[ci]
                z_c = ztc[ci]
                for m in range(3):
                    nc.tensor.matmul(
                        psg[m][:],
                        lhsT=z_c[:, kk * 384 + m * 128:kk * 384 + m * 128 + 128],
                        rhs=z_c[:, kk * 384 + m * 128:(kk + 1) * 384],
                        start=(k == 0),
                        stop=(k == KT - 1),
                    )

            # single AllReduce of the packed upper blocks (one collective;
            # per-collective latency here is ~25us regardless of size)
            g = sb.tile([128, 768], BF16)
            for m in range(3):
                nc.vector.tensor_copy(g[:, UPO[m]:UPO[m] + UPW[m]], psg[m][:])
            cc_in = dram.tile([128, 768], BF16)
            nc.sync.dma_start(cc_in[:], g[:])
            cc_out = dram.tile([128, 768], BF16, addr_space="Shared")
            nc.gpsimd.collective_compute(
                "AllReduce", AOp.add,
                replica_groups=[list(range(N_CORES))],
                ins=[cc_in.opt()], outs=[cc_out.opt()],
            )
            gu = sb.tile([128, 768], F32)
            nc.gpsimd.dma_start(gu[:], cc_out[:])

            # norm chain straight off the packed upper buffer: the diagonal
            # block of row-block m is the first 128 cols of its span, so the
            # diag extraction is a 128-wide window and runs while the mirror
            # transposes below reconstruct the dense layout
            inv, inv10 = [], []
            scr = sb.tile([128, 384], F32)
            for m in range(3):
                eoff = m * 384 + m * 128  # eye(128) block within the mask
                nrm2_m = sb.tile([128, 1], F32, name=f"nrm2_{m}")
                nc.vector.scalar_tensor_tensor(
                    out=scr[:, 0:128], in0=gu[:, UPO[m]:UPO[m] + 128],
                    scalar=1.0, in1=eye[:, eoff:eoff + 128],
                    op0=AOp.mult, op1=AOp.mult, accum_out=nrm2_m[:])
                nrm_m = sb.tile([128, 1], F32, name=f"nrm_{m}")
                nc.scalar.sqrt(nrm_m[:], nrm2_m[:])
                inv_m = sb.tile([128, 1], F32, name=f"inv_{m}")
                nc.vector.reciprocal(inv_m[:], nrm_m[:])
                inv10_m = sb.tile([128, 1], F32, name=f"inv10_{m}")
                nc.vector.tensor_scalar_mul(inv10_m[:], inv_m[:], 1.0 / TEMP)
                inv.append(inv_m)
                inv10.append(inv10_m)

            # reconstruct the dense [128, 3*384] row-block layout: direct
            # copies for stored blocks, PE transposes for the mirrored ones
            gr = sb.tile([128, 1152], F32)
            nc.vector.tensor_copy(gr[:, 0:384], gu[:, 0:384])
            nc.vector.tensor_copy(gr[:, 512:768], gu[:, 384:640])
            nc.vector.tensor_copy(gr[:, 1024:1152], gu[:, 640:768])
            with tc.tile_pool(name="ps2", bufs=2, space="PSUM") as ps2:
                for srcc, dst in ((128, 384), (256, 768), (512, 896)):
                    ps_t = ps2.tile([128, 128], F32, name="ps_t")
                    nc.tensor.transpose(ps_t[:], gu[:, srcc:srcc + 128],
                                        eye[:, 0:128])
                    nc.vector.tensor_copy(gr[:, dst:dst + 128], ps_t[:])

            # replicate 1/nrm as a row across partitions: bf16 PE transpose +
            # bf16 K=1 matmul (single-pass, vs 2-pass fp32)
            eye_bf = sb.tile([128, 128], BF16)
            nc.vector.tensor_copy(eye_bf[:], eye[:, 0:128])
            ps_r = ps.tile([1, 384], BF16)
            for m in range(3):
                inv_bf_m = sb.tile([128, 1], BF16, name=f"inv_bf_{m}")
                nc.vector.tensor_copy(inv_bf_m[:], inv[m][:])
                nc.tensor.transpose(ps_r[:, m * 128:(m + 1) * 128], inv_bf_m[:],
                                    eye_bf[:])
            invrow = sb.tile([1, 384], BF16)
            nc.vector.tensor_copy(invrow[:], ps_r[:])
            ones_row = sb.tile([1, 128], BF16)
            nc.vector.memset(ones_row[:], 1.0)
            ps_b = ps.tile([128, 384], F32)
            nc.tensor.matmul(ps_b[:], lhsT=ones_row[:], rhs=invrow[:],
                             start=True, stop=True)

            ones_col = sb.tile([128, 1], F32)
            nc.vector.memset(ones_col[:], 1.0)
            ps_s = ps.tile([1, 1], F32)

            for m in range(3):
                msl = slice(m * 384, (m + 1) * 384)
                # E = exp((1/T) * inv_row ⊙ G ⊙ inv_col): col scale on DVE,
                # row scale + 1/T folded into the ACT Exp per-partition scale
                nc.vector.tensor_tensor(gr[:, msl], gr[:, msl], ps_b[:], AOp.mult)
                nc.scalar.activation(gr[:, msl], gr[:, msl], AF.Exp, scale=inv10[m][:])
                dsum_m = sb.tile([128, 1], F32, name=f"dsum_{m}")
                nc.vector.scalar_tensor_tensor(
                    out=scr[:], in0=gr[:, msl], scalar=1.0, in1=neg[:, msl],
                    op0=AOp.mult, op1=AOp.mult, accum_out=dsum_m[:])
                # each shifted-diagonal mask has one nonzero per row inside
                # a 128-col window for the non-wrapping (mask, m) pairs
                D192W = (192, None, 64)
                D16W = (16, 144, None)
                D16MW = (None, 112, 240)

                def diag_stt(mask, w, name_):
                    v = sb.tile([128, 1], F32, name=name_)
                    if w is None:
                        nc.vector.scalar_tensor_tensor(
                            out=scr[:], in0=gr[:, msl], scalar=1.0,
                            in1=mask[:, msl],
                            op0=AOp.mult, op1=AOp.mult, accum_out=v[:])
                    else:
                        a = m * 384 + w
                        nc.vector.scalar_tensor_tensor(
                            out=scr[:, 0:128], in0=gr[:, a:a + 128],
                            scalar=1.0, in1=mask[:, a:a + 128],
                            op0=AOp.mult, op1=AOp.mult, accum_out=v[:])
                    return v

                v192_m = diag_stt(d192, D192W[m], f"v192_{m}")
                v16_m = diag_stt(d16, D16W[m], f"v16_{m}")
                v16m_m = diag_stt(d16m, D16MW[m], f"v16m_{m}")

                pack_m = sb.tile([128, 5], F32, name=f"pack_{m}")
                nc.vector.tensor_tensor(pack_m[:, 0:1], v192_m[:], dsum_m[:], AOp.add)
                nc.vector.tensor_tensor(pack_m[:, 1:2], v16_m[:], dsum_m[:], AOp.add)
                nc.vector.tensor_tensor(pack_m[:, 2:3], v16m_m[:], dsum_m[:], AOp.add)
                nc.vector.tensor_copy(pack_m[:, 3:4], v192_m[:])
                nc.vector.tensor_copy(pack_m[:, 4:5], v16_m[:])
                nc.scalar.activation(pack_m[:], pack_m[:], AF.Ln)
                scr5 = sb.tile([128, 5], F32, name="scr5")
                wsum_m = sb.tile([128, 1], F32, name=f"wsum_{m}")
                nc.vector.scalar_tensor_tensor(
                    out=scr5[:], in0=pack_m[:], scalar=1.0, in1=wts[:],
                    op0=AOp.mult, op1=AOp.mult, accum_out=wsum_m[:])
                nc.tensor.matmul(ps_s[:], lhsT=wsum_m[:], rhs=ones_col[:],
                                 start=(m == 0), stop=(m == 2))

            res = sb.tile([1, 1], F32)
            nc.vector.tensor_scalar_mul(res[:], ps_s[:], 1.0 / DENOM)
            nc.sync.dma_start(out_d.ap()[:], res[:])
    nc.compile()
    return nc


def _host_consts():
    i = np.arange(N)
    rem = i % PD

    def lay(m):
        return np.ascontiguousarray(
            m.reshape(3, 128, N).transpose(1, 0, 2).reshape(128, 3 * N)
        ).astype(np.float32)

    eye = lay(i[:, None] == i[None, :])
    neg = lay(rem[:, None] != rem[None, :])
    d192 = lay(i[None, :] == (i[:, None] + 192) % N)
    d16 = lay(i[None, :] == (i[:, None] + 16) % N)
    d16m = lay(i[None, :] == (i[:, None] - 16) % N)
    wts = np.broadcast_to(
        np.array([2.0, 1.0, 1.0, -2.0, -2.0], np.float32), (128, 5)
    ).copy()
    return eye, neg, d192, d16, d16m, wts


def _heal_device():
    """Best-effort recovery if a previous process left a NeuronCore in the
    NRT_EXEC_UNIT_UNRECOVERABLE state (axon terminal keeps it wedged
    across client processes otherwise). Harmless on a healthy device."""
    try:
        import ctypes
        import jax
        jax.devices()
        lib = ctypes.CDLL("/opt/axon/libaxon_pjrt.so")
        lib.axon_reset.restype = ctypes.c_int64
        lib.axon_reset()
    except Exception:
        pass


def kernel(reg_pred, mean_representations):
    global LAST_RESULTS
    if "healed" not in _CACHE:
        _heal_device()
        _CACHE["healed"] = True
    X = np.asarray(reg_pred, dtype=np.float32).reshape(N, D)
    M4 = np.asarray(mean_representations, dtype=np.float32).reshape(4, D)
    ds16 = np.array([0, 0, 0, 0, 1, 1, 1, 1, 2, 2, 2, 2, 3, 3, 3, 3])
    M16 = M4[ds16]  # [16, D]

    eye, neg, d192, d16, d16m, wts = _host_consts()

    XT = X.T  # [D, N] view
    MT = M16.T  # [D, 16] view
    in_maps = []
    for c in range(N_CORES):
        sl = slice(c * DL, (c + 1) * DL)
        xt = XT[sl].reshape(KT, 128, N).transpose(1, 0, 2) \
            .reshape(128, KT * N).astype(ml_dtypes.bfloat16)
        mt = MT[sl].reshape(KT, 128, 16).transpose(1, 0, 2) \
            .reshape(128, KT * 16).astype(ml_dtypes.bfloat16)
        in_maps.append({
            "xt": xt, "mt": mt, "eye": eye, "neg": neg,
            "d192": d192, "d16": d16, "d16m": d16m, "wts": wts,
        })

    if "nc" not in _CACHE:
        _CACHE["nc"] = _build_module()
    nc = _CACHE["nc"]

    res = bass_utils.run_bass_kernel_spmd(
        nc, in_maps, core_ids=list(range(N_CORES))
    )
    LAST_RESULTS = res
    return np.asarray(res.results[0]["out"][0, 0], dtype=np.float32)


# revision 24
# speedup vs baseline: 1.1299x; 1.1299x over previous
"""Trainium2 Bass kernel for nn_DemeanedGlobalLossD.

Strategy: shard the feature dim D=73728 over 8 cores (9216 each). Each core
computes the upper block-rows of a partial Gram matrix G_c = Z_c @ Z_c.T
([384,384], bf16 matmuls, fp32 accum) of the demeaned features; one bf16
AllReduce sums the partials and PE transposes mirror the lower blocks
(G is exactly symmetric). The row norms are sqrt(diag(G)), so the
normalize step needs no extra reduction. The loss tail (cosine scaling,
exp, masked negative sums, pair log terms) runs replicated on every core
on the tiny 384x384 matrix.

The 4 pair families of the reference all use partner offsets +16 / +192
(mod 384), and S is symmetric, so the 768 pair terms reduce to elementwise
logs over three shifted diagonals of E = exp(S/T):
  total = sum_i [ 2*log(v192+Ds) + log(v16+Ds) + log(v16m+Ds)
                  - 2*log(v192) - 2*log(v16) ](i)
with v192[i]=E[i,(i+192)%384], v16[i]=E[i,(i+16)%384], v16m[i]=E[i,(i-16)%384],
Ds = masked (j%16 != i%16) row sums of E.  loss = total / 576.
"""
import numpy as np
import ml_dtypes

import concourse.bass as bass
import concourse.bacc as bacc
import concourse.tile as tile
import concourse.mybir as mybir
from concourse import bass_utils

F32 = mybir.dt.float32
BF16 = mybir.dt.bfloat16

N = 384
D = 73728
N_CORES = 8
DL = D // N_CORES          # 9216 features per core
KT = DL // 128             # 72 k-tiles per core
CHUNK_KT = (2, 10, 10, 10, 10, 10, 10, 10)   # k-tiles per xt chunk
NCHUNK = len(CHUNK_KT)
PD = 16
TEMP = 0.1
EPS = 1e-6
DENOM = 576.0              # N_TRANSFORMS * 3 * BS

_CACHE = {}
LAST_RESULTS = None


def _build_module():
    nc = bacc.Bacc("TRN2", target_bir_lowering=False, debug=False,
                   num_devices=N_CORES)
    xt_d = nc.dram_tensor("xt", [128, KT * 384], BF16, kind="ExternalInput")
    mt_d = nc.dram_tensor("mt", [128, KT * 16], BF16, kind="ExternalInput")
    eye_d = nc.dram_tensor("eye", [128, 1152], F32, kind="ExternalInput")
    neg_d = nc.dram_tensor("neg", [128, 1152], F32, kind="ExternalInput")
    d192_d = nc.dram_tensor("d192", [128, 1152], F32, kind="ExternalInput")
    d16_d = nc.dram_tensor("d16", [128, 1152], F32, kind="ExternalInput")
    d16m_d = nc.dram_tensor("d16m", [128, 1152], F32, kind="ExternalInput")
    wts_d = nc.dram_tensor("wts", [128, 5], F32, kind="ExternalInput")
    out_d = nc.dram_tensor("out", [1, 1], F32, kind="ExternalOutput")

    AOp = mybir.AluOpType
    AF = mybir.ActivationFunctionType
    chunk_of_k = []
    for ci, n in enumerate(CHUNK_KT):
        chunk_of_k += [ci] * n
    chunk_base = [sum(CHUNK_KT[:ci]) for ci in range(NCHUNK)]

    with tile.TileContext(nc) as tc:
        with (
            tc.tile_pool(name="sb", bufs=1) as sb,
            tc.tile_pool(name="ps", bufs=1, space="PSUM") as ps,
            tc.tile_pool(name="dram", bufs=1, space="DRAM") as dram,
        ):
            # mean via the scalar HWDGE queue so it races ahead of the x
            # chunks on the sync queue; masks follow on the same queue
            # (they are needed only after the AllReduce)
            mt = sb.tile([128, KT * 16], BF16)
            nc.scalar.dma_start(mt[:], mt_d.ap()[:])
            # x chunks as separate tiles for chunk-granular deps
            xtc = []
            for ci in range(NCHUNK):
                cw = CHUNK_KT[ci] * 384
                co = chunk_base[ci] * 384
                x_c = sb.tile([128, cw], BF16, name=f"xt{ci}")
                nc.sync.dma_start(x_c[:], xt_d.ap()[:, co:co + cw])
                xtc.append(x_c)
            eye = sb.tile([128, 1152], F32)
            nc.scalar.dma_start(eye[:], eye_d.ap()[:])
            neg = sb.tile([128, 1152], F32)
            nc.scalar.dma_start(neg[:], neg_d.ap()[:])
            d192 = sb.tile([128, 1152], F32)
            nc.scalar.dma_start(d192[:], d192_d.ap()[:])
            d16 = sb.tile([128, 1152], F32)
            nc.scalar.dma_start(d16[:], d16_d.ap()[:])
            d16m = sb.tile([128, 1152], F32)
            nc.scalar.dma_start(d16m[:], d16m_d.ap()[:])
            wts = sb.tile([128, 5], F32)
            nc.scalar.dma_start(wts[:], wts_d.ap()[:])

            # preload the ACT tables (Sqrt/Exp/Ln) while DMAs stream so the
            # table loads don't land in the post-AllReduce critical path
            tbl = sb.tile([1, 1], F32)
            nc.vector.memset(tbl[:], 1.0)
            nc.scalar.sqrt(tbl[:], tbl[:])
            nc.scalar.activation(tbl[:], tbl[:], AF.Exp)
            nc.scalar.activation(tbl[:], tbl[:], AF.Ln)

            # demean into per-chunk zt tiles (broadcast 24 reps of the 16 means)
            ztc = []
            for ci in range(NCHUNK):
                z_c = sb.tile([128, CHUNK_KT[ci] * 384], BF16, name=f"zt{ci}")
                ztc.append(z_c)
            for k in range(KT):
                ci = chunk_of_k[k]
                kk = k - chunk_base[ci]
                in1 = mt[:, k * 16:(k + 1) * 16].unsqueeze(1).broadcast_to([128, 24, 16])
                o3 = ztc[ci][:, kk * 384:(kk + 1) * 384].rearrange("p (a b) -> p a b", b=16)
                i3 = xtc[ci][:, kk * 384:(kk + 1) * 384].rearrange("p (a b) -> p a b", b=16)
                nc.vector.tensor_tensor(o3, i3, in1, AOp.subtract)

            # partial Gram, upper block-rows only (G is exactly symmetric):
            # m=0 -> abs cols 0:384, m=1 -> 128:384, m=2 -> 256:384.
            # k-outer so PE paces the DMA stream once.
            UPW = (384, 256, 128)          # upper widths per block-row
            UPO = (0, 384, 640)            # offsets in the packed buffer
            psg = [ps.tile([128, UPW[m]], F32, name=f"psg{m}") for m in range(3)]

            def gram_mm(k, m):
                ci = chunk_of_k[k]
                kk = k - chunk_base[ci]
                z_c = ztc[ci]
                nc.tensor.matmul(
                    psg[m][:],
                    lhsT=z_c[:, kk * 384 + m * 128:kk * 384 + m * 128 + 128],
                    rhs=z_c[:, kk * 384 + m * 128:(kk + 1) * 384],
                    start=(k == 0),
                    stop=(k == KT - 1),
                )

            LASTC = KT - CHUNK_KT[-1]  # last chunk handled m-major below
            for k in range(LASTC):
                for m in range(3):
                    gram_mm(k, m)

            # single AllReduce of the packed upper blocks (one collective;
            # per-collective latency here is ~25us regardless of size).
            # The last chunk runs m-major (widest block last) so each PSUM
            # block evacuates and uploads while the rest still computes.
            g = sb.tile([128, 768], BF16)
            cc_in = dram.tile([128, 768], BF16)
            for m in (2, 1, 0):
                for k in range(LASTC, KT):
                    gram_mm(k, m)
                nc.vector.tensor_copy(g[:, UPO[m]:UPO[m] + UPW[m]], psg[m][:])
                nc.sync.dma_start(cc_in[:, UPO[m]:UPO[m] + UPW[m]],
                                  g[:, UPO[m]:UPO[m] + UPW[m]])
            cc_out = dram.tile([128, 768], BF16, addr_space="Shared")
            nc.gpsimd.collective_compute(
                "AllReduce", AOp.add,
                replica_groups=[list(range(N_CORES))],
                ins=[cc_in.opt()], outs=[cc_out.opt()],
            )
            gu = sb.tile([128, 768], F32)
            nc.gpsimd.dma_start(gu[:], cc_out[:])

            # norm chain straight off the packed upper buffer: the diagonal
            # block of row-block m is the first 128 cols of its span, so the
            # diag extraction is a 128-wide window and runs while the mirror
            # transposes below reconstruct the dense layout
            inv, inv10 = [], []
            scr = sb.tile([128, 384], F32)
            for m in range(3):
                eoff = m * 384 + m * 128  # eye(128) block within the mask
                nrm2_m = sb.tile([128, 1], F32, name=f"nrm2_{m}")
                nc.vector.scalar_tensor_tensor(
                    out=scr[:, 0:128], in0=gu[:, UPO[m]:UPO[m] + 128],
                    scalar=1.0, in1=eye[:, eoff:eoff + 128],
                    op0=AOp.mult, op1=AOp.mult, accum_out=nrm2_m[:])
                nrm_m = sb.tile([128, 1], F32, name=f"nrm_{m}")
                nc.scalar.sqrt(nrm_m[:], nrm2_m[:])
                inv_m = sb.tile([128, 1], F32, name=f"inv_{m}")
                nc.vector.reciprocal(inv_m[:], nrm_m[:])
                inv10_m = sb.tile([128, 1], F32, name=f"inv10_{m}")
                nc.vector.tensor_scalar_mul(inv10_m[:], inv_m[:], 1.0 / TEMP)
                inv.append(inv_m)
                inv10.append(inv10_m)

            # reconstruct the dense [128, 3*384] row-block layout: direct
            # copies for stored blocks, PE transposes for the mirrored ones
            gr = sb.tile([128, 1152], F32)
            nc.vector.tensor_copy(gr[:, 0:384], gu[:, 0:384])
            nc.vector.tensor_copy(gr[:, 512:768], gu[:, 384:640])
            nc.vector.tensor_copy(gr[:, 1024:1152], gu[:, 640:768])
            with tc.tile_pool(name="ps2", bufs=2, space="PSUM") as ps2:
                for srcc, dst in ((128, 384), (256, 768), (512, 896)):
                    ps_t = ps2.tile([128, 128], F32, name="ps_t")
                    nc.tensor.transpose(ps_t[:], gu[:, srcc:srcc + 128],
                                        eye[:, 0:128])
                    nc.vector.tensor_copy(gr[:, dst:dst + 128], ps_t[:])

            # replicate 1/nrm as a row across partitions: bf16 PE transpose +
            # bf16 K=1 matmul (single-pass, vs 2-pass fp32)
            eye_bf = sb.tile([128, 128], BF16)
            nc.vector.tensor_copy(eye_bf[:], eye[:, 0:128])
            ps_r = ps.tile([1, 384], BF16)
            for m in range(3):
                inv_bf_m = sb.tile([128, 1], BF16, name=f"inv_bf_{m}")
                nc.vector.tensor_copy(inv_bf_m[:], inv[m][:])
                nc.tensor.transpose(ps_r[:, m * 128:(m + 1) * 128], inv_bf_m[:],
                                    eye_bf[:])
            invrow = sb.tile([1, 384], BF16)
            nc.vector.tensor_copy(invrow[:], ps_r[:])
            ones_row = sb.tile([1, 128], BF16)
            nc.vector.memset(ones_row[:], 1.0)
            ps_b = ps.tile([128, 384], F32)
            nc.tensor.matmul(ps_b[:], lhsT=ones_row[:], rhs=invrow[:],
                             start=True, stop=True)

            ones_col = sb.tile([128, 1], F32)
            nc.vector.memset(ones_col[:], 1.0)
            ps_s = ps.tile([1, 1], F32)

            for m in range(3):
                msl = slice(m * 384, (m + 1) * 384)
                # E = exp((1/T) * inv_row ⊙ G ⊙ inv_col): col scale on DVE,
                # row scale + 1/T folded into the ACT Exp per-partition scale
                nc.vector.tensor_tensor(gr[:, msl], gr[:, msl], ps_b[:], AOp.mult)
                nc.scalar.activation(gr[:, msl], gr[:, msl], AF.Exp, scale=inv10[m][:])
                dsum_m = sb.tile([128, 1], F32, name=f"dsum_{m}")
                nc.vector.scalar_tensor_tensor(
                    out=scr[:], in0=gr[:, msl], scalar=1.0, in1=neg[:, msl],
                    op0=AOp.mult, op1=AOp.mult, accum_out=dsum_m[:])
                # each shifted-diagonal mask has one nonzero per row inside
                # a 128-col window for the non-wrapping (mask, m) pairs
                D192W = (192, None, 64)
                D16W = (16, 144, None)
                D16MW = (None, 112, 240)

                def diag_stt(mask, w, name_):
                    v = sb.tile([128, 1], F32, name=name_)
                    if w is None:
                        nc.vector.scalar_tensor_tensor(
                            out=scr[:], in0=gr[:, msl], scalar=1.0,
                            in1=mask[:, msl],
                            op0=AOp.mult, op1=AOp.mult, accum_out=v[:])
                    else:
                        a = m * 384 + w
                        nc.vector.scalar_tensor_tensor(
                            out=scr[:, 0:128], in0=gr[:, a:a + 128],
                            scalar=1.0, in1=mask[:, a:a + 128],
                            op0=AOp.mult, op1=AOp.mult, accum_out=v[:])
                    return v

                v192_m = diag_stt(d192, D192W[m], f"v192_{m}")
                v16_m = diag_stt(d16, D16W[m], f"v16_{m}")
                v16m_m = diag_stt(d16m, D16MW[m], f"v16m_{m}")

                pack_m = sb.tile([128, 5], F32, name=f"pack_{m}")
                nc.vector.tensor_tensor(pack_m[:, 0:1], v192_m[:], dsum_m[:], AOp.add)
                nc.vector.tensor_tensor(pack_m[:, 1:2], v16_m[:], dsum_m[:], AOp.add)
                nc.vector.tensor_tensor(pack_m[:, 2:3], v16m_m[:], dsum_m[:], AOp.add)
                nc.vector.tensor_copy(pack_m[:, 3:4], v192_m[:])
                nc.vector.tensor_copy(pack_m[:, 4:5], v16_m[:])
                nc.scalar.activation(pack_m[:], pack_m[:], AF.Ln)
                scr5 = sb.tile([128, 5], F32, name="scr5")
                wsum_m = sb.tile([128, 1], F32, name=f"wsum_{m}")
                nc.vector.scalar_tensor_tensor(
                    out=scr5[:], in0=pack_m[:], scalar=1.0, in1=wts[:],
                    op0=AOp.mult, op1=AOp.mult, accum_out=wsum_m[:])
                nc.tensor.matmul(ps_s[:], lhsT=wsum_m[:], rhs=ones_col[:],
                                 start=(m == 0), stop=(m == 2))

            res = sb.tile([1, 1], F32)
            nc.vector.tensor_scalar_mul(res[:], ps_s[:], 1.0 / DENOM)
            nc.sync.dma_start(out_d.ap()[:], res[:])
    nc.compile()
    return nc


def _host_consts():
    i = np.arange(N)
    rem = i % PD

    def lay(m):
        return np.ascontiguousarray(
            m.reshape(3, 128, N).transpose(1, 0, 2).reshape(128, 3 * N)
        ).astype(np.float32)

    eye = lay(i[:, None] == i[None, :])
    neg = lay(rem[:, None] != rem[None, :])
    d192 = lay(i[None, :] == (i[:, None] + 192) % N)
    d16 = lay(i[None, :] == (i[:, None] + 16) % N)
    d16m = lay(i[None, :] == (i[:, None] - 16) % N)
    wts = np.broadcast_to(
        np.array([2.0, 1.0, 1.0, -2.0, -2.0], np.float32), (128, 5)
    ).copy()
    return eye, neg, d192, d16, d16m, wts


def _heal_device():
    """Best-effort recovery if a previous process left a NeuronCore in the
    NRT_EXEC_UNIT_UNRECOVERABLE state (axon terminal keeps it wedged
    across client processes otherwise). Harmless on a healthy device."""
    try:
        import ctypes
        import jax
        jax.devices()
        lib = ctypes.CDLL("/opt/axon/libaxon_pjrt.so")
        lib.axon_reset.restype = ctypes.c_int64
        lib.axon_reset()
    except Exception:
        pass


def kernel(reg_pred, mean_representations):
    global LAST_RESULTS
    if "healed" not in _CACHE:
        _heal_device()
        _CACHE["healed"] = True
    X = np.asarray(reg_pred, dtype=np.float32).reshape(N, D)
    M4 = np.asarray(mean_representations, dtype=np.float32).reshape(4, D)
    ds16 = np.array([0, 0, 0, 0, 1, 1, 1, 1, 2, 2, 2, 2, 3, 3, 3, 3])
    M16 = M4[ds16]  # [16, D]

    eye, neg, d192, d16, d16m, wts = _host_consts()

    XT = X.T  # [D, N] view
    MT = M16.T  # [D, 16] view
    in_maps = []
    for c in range(N_CORES):
        sl = slice(c * DL, (c + 1) * DL)
        xt = XT[sl].reshape(KT, 128, N).transpose(1, 0, 2) \
            .reshape(128, KT * N).astype(ml_dtypes.bfloat16)
        mt = MT[sl].reshape(KT, 128, 16).transpose(1, 0, 2) \
            .reshape(128, KT * 16).astype(ml_dtypes.bfloat16)
        in_maps.append({
            "xt": xt, "mt": mt, "eye": eye, "neg": neg,
            "d192": d192, "d16": d16, "d16m": d16m, "wts": wts,
        })

    if "nc" not in _CACHE:
        _CACHE["nc"] = _build_module()
    nc = _CACHE["nc"]

    res = bass_utils.run_bass_kernel_spmd(
        nc, in_maps, core_ids=list(range(N_CORES))
    )
    LAST_RESULTS = res
    return np.asarray(res.results[0]["out"][0, 0], dtype=np.float32)
